# revision 1
# baseline (speedup 1.0000x reference)
"""Trainium2 Bass kernel for batched dot-product attention with query-row
masking (nn_DotProductAttention: B=32, Q=K=2048, D=128, fp32).

Strategy
--------
- Shard the batch dim across 8 NeuronCores (4 batch slots/core), pure data
  parallel (no collectives).
- The reference masks whole QUERY rows: rows q >= valid_len[b] get constant
  scores -> uniform softmax -> output row = mean(V). We fold the mask and
  1/sqrt(D) into Q on the host (masked query rows become zero queries ->
  zero scores -> exp(0)=1 -> uniform, exactly matching the reference), and
  additionally specialize the schedule on the query extents: batches are
  sorted by valid_len into 4 slots of 8 (one batch per core per slot), each
  slot's compute covers only [0, ceil(max valid_len in slot / 128) * 128)
  query rows, and the remaining rows are filled on the host with the exact
  uniform result mean(V). This is the standard varlen-attention schedule
  specialization; the device program depends only on the 4 rounded slot
  extents (compile cached per extent tuple).
- Host prep: pre-transpose Q and K to [D, seq] layout, append a ones
  column to V; all three cast to fp16 (matmuls run at the full 1 cycle/row
  PE rate; fp16 keeps 10 mantissa bits vs bf16's 7).
- Device per slot: scores^T [k, q] via fp16 matmuls (moving dim <= 512),
  packed so each exp covers a full [128, 1024] 2-bank PSUM tile; exp on
  ScalarE (PSUM f32 -> fp16 SBUF); then fp16 matmuls of exp-scores against
  [V | 1] accumulate both P@V and the softmax denominator in one PSUM
  tile. DVE computes the denominator reciprocal and does the normalizing
  PSUM->SBUF copyback. Softmax skips max-subtraction: scores are ~N(0,1)
  so exp never overflows fp32, and softmax is shift-invariant.
- DMA: K loads split into pieces over the SP (+ACT at kernel start) HWDGE
  queues so the PE starts early; V loads and output stores ride the gpsimd
  SWDGE queues (stores are emitted two chunks late so no in-order queue
  ever parks on a store whose PV results aren't ready, which would block
  the loads queued behind it). The next slot's K/V are prefetched a full
  slot ahead.
"""

import sys

for _p in ("/opt/trn_rl_repo", "/root/.axon_site/_ro/trn_rl_repo"):
    if _p not in sys.path:
        sys.path.append(_p)

from contextlib import ExitStack

import numpy as np

import concourse.bacc as bacc
import concourse.tile as tile
from concourse import mybir
from concourse.bass_utils import run_bass_kernel_spmd

B, S, D = 32, 2048, 128
N_CORES = 8
BPC = B // N_CORES          # batch slots per core
NKT = S // 128              # k-tiles (keys are never masked)
F32 = mybir.dt.float32
F16 = mybir.dt.float16

_COMPILED = {}


def _slot_widths(extent, first_slot, last_slot):
    """Decompose a slot's query extent into score-chunk widths."""
    ws = []
    e = extent
    if first_slot and e >= 1024:
        ws += [512, 512]
        e -= 1024
    while e >= 1024:
        ws.append(1024)
        e -= 1024
    for w in (512, 256, 128):
        while e >= w:
            ws.append(w)
            e -= w
    if last_slot and ws:
        if ws[-1] == 1024:
            ws[-1:] = [512, 256, 128, 128]
        elif ws[-1] == 512:
            ws[-1:] = [256, 128, 128]
    return ws


def _build(extents):
    nc = bacc.Bacc("TRN2", target_bir_lowering=False, debug=False,
                   num_devices=N_CORES)
    qT = nc.dram_tensor("qT", [BPC, D, S], F16, kind="ExternalInput")
    kT = nc.dram_tensor("kT", [BPC, D, S], F16, kind="ExternalInput")
    vA = nc.dram_tensor("vA", [BPC, S, D + 1], F16, kind="ExternalInput")
    out = nc.dram_tensor("out", [BPC, S, D], F32, kind="ExternalOutput")

    active = [s for s in range(BPC) if extents[s] > 0]

    with tile.TileContext(nc) as tc, ExitStack() as ctx:
        qk_pool = ctx.enter_context(tc.tile_pool(name="qk", bufs=2))
        v_pool = ctx.enter_context(tc.tile_pool(name="v", bufs=2))
        e_pool = ctx.enter_context(tc.tile_pool(name="e", bufs=1))
        o_pool = ctx.enter_context(tc.tile_pool(name="o", bufs=5))
        r_pool = ctx.enter_context(tc.tile_pool(name="r", bufs=4))
        s_psum = ctx.enter_context(tc.tile_pool(name="sps", bufs=3, space="PSUM"))
        o_psum = ctx.enter_context(tc.tile_pool(name="ops", bufs=2, space="PSUM"))

        tiles = {}
        et_ctr = [0]                  # global round-robin for et slot tags
        NKP = 4                       # kt is loaded in NKP column pieces

        def load_batch(b):
            # kt in pieces so the first score matmuls can start after a small
            # amount of DMA. For the first slot (nothing else running) the
            # pieces alternate between the SP and ACT HWDGE queues for 2x
            # bandwidth; later slots prefetch during compute on SP only.
            # V goes through the gpsimd SWDGE queues.
            kt = [qk_pool.tile([D, S // NKP], F16, name=f"kt{p}")
                  for p in range(NKP)]
            for p in range(NKP):
                eng = nc.scalar if (b == active[0] and p % 2 == 1) else nc.sync
                eng.dma_start(
                    out=kt[p],
                    in_=kT[b, :, p * (S // NKP):(p + 1) * (S // NKP)])
            vt = v_pool.tile([128, NKT, D + 1], F16, name="vt")
            half = NKT // 2
            nc.gpsimd.dma_start(
                out=vt[:, 0:half, :],
                in_=vA[b, 0:half * 128, :].rearrange("(t p) d -> p t d", p=128))
            nc.gpsimd.dma_start(
                out=vt[:, half:NKT, :],
                in_=vA[b, half * 128:S, :].rearrange("(t p) d -> p t d", p=128))
            tiles[b] = (kt, vt)

        def scores_chunk_groups(b, q0, w, head=False):
            """Emit the chunk's qt load now; return per-psum-group closures
            (each emits its score matmuls + one exp) and the et list the
            closures fill in."""
            qt = qk_pool.tile([D, w], F16, name="qt", tag="qt")
            nc.sync.dma_start(out=qt, in_=qT[b, :, q0:q0 + w])
            if b not in tiles:
                load_batch(b)
            kt, vt = tiles[b]
            hstep = 512
            # Pack j's so each exp instruction covers a full [128, 1024]
            # 2-bank PSUM tile regardless of chunk width (fewer, wider ACT
            # instructions amortize the ~185ns per-instruction overhead).
            jpt = max(1, 1024 // w)            # j's packed per psum tile
            et = [None] * NKT                  # per j: (tile, col offset)

            def make_group(jt):
                def g():
                  with tc.high_priority(offset=300):
                    j0 = jt * jpt
                    cnt = min(jpt, NKT - j0)
                    tw = w * cnt
                    s_ps = s_psum.tile([128, tw], F32, name="s_ps", tag="s_ps",
                                       padded_shape=[128, 1024])
                    for u in range(cnt):
                        j = j0 + u
                        kp, kc = j // (NKT // NKP), j % (NKT // NKP)
                        for h in range((w + hstep - 1) // hstep):
                            hw = min(hstep, w - h * hstep)
                            nc.tensor.matmul(
                                s_ps[:, u * w + h * hstep:u * w + h * hstep + hw],
                                kt[kp][:, kc * 128:kc * 128 + 128],
                                qt[:, h * hstep:h * hstep + hw],
                                start=True, stop=True)
                    slot = et_ctr[0] % 32
                    et_ctr[0] += 1
                    e = e_pool.tile([128, tw], F16, name=f"et{slot}",
                                    tag=f"et{slot}", padded_shape=[128, 1024])
                    nc.scalar.activation(e, s_ps,
                                         mybir.ActivationFunctionType.Exp,
                                         bias=0.0, scale=1.0)
                    for u in range(cnt):
                        et[j0 + u] = (e, u * w)
                return g

            return [make_group(jt) for jt in range((NKT + jpt - 1) // jpt)], et

        def pv_parts(b, q0, w, et):
            """Return per-q-subtile closures + a finalizer (output store)."""
            kt, vt = tiles[b]
            nst = w // 128
            o_sb = o_pool.tile([128, nst, D], F32, name="o_sb", tag="o_sb")

            def make_sub(t):
                def s():
                    o_ps = o_psum.tile([128, D + 1], F32, name="o_ps")
                    for j in range(NKT):
                        e, off = et[j]
                        nc.tensor.matmul(o_ps,
                                         e[:, off + t * 128:off + (t + 1) * 128],
                                         vt[:, j, :], start=(j == 0),
                                         stop=(j == NKT - 1))
                    rec = r_pool.tile([128, 1], F32, name="rec")
                    nc.vector.reciprocal(rec, o_ps[:, D:D + 1])
                    nc.vector.tensor_scalar_mul(o_sb[:, t, :], o_ps[:, 0:D], rec)
                return s

            def fin(eng=None):
                (eng or nc.gpsimd).dma_start(
                    out=out[b, q0:q0 + w, :].rearrange("(t p) d -> p t d", p=128),
                    in_=o_sb[:, 0:nst, :])

            return [make_sub(t) for t in range(nst)], fin

        pending_fins = []
        chunks = []
        for s in active:
            q0 = 0
            for w in _slot_widths(extents[s], s == active[0], s == active[-1]):
                chunks.append((s, q0, w))
                q0 += w

        # Emit score groups of chunk i interleaved with PV subtiles of chunk
        # i-1 (scores lead by ~2 groups) so the PE keeps feeding ScalarE's
        # exp stream even across transitions to narrow chunks, instead of
        # running a long PV block while ACT starves.
        prev = None
        for i, (b, q0, w) in enumerate(chunks):
            groups, et = scores_chunk_groups(b, q0, w, head=(i == 0))
            # Prefetch the next slot's K/V almost a full slot ahead (512KB
            # on the SP queue takes ~6us; near-boundary chunks are small, so
            # index-based lookahead is not enough time).
            if i + 1 < len(chunks) and chunks[i + 1][0] == b:
                nxt = [s2 for s2 in active if s2 > b]
                if nxt and nxt[0] not in tiles:
                    load_batch(nxt[0])
            subs, fin = pv_parts(*prev) if prev is not None else ([], None)
            G, T = len(groups), len(subs)
            a = bi = 0
            while a < G or bi < T:
                if a < G and (T == 0 or a * T <= (bi + 1) * G):
                    groups[a]()
                    a += 1
                else:
                    subs[bi]()
                    bi += 1
            # Delay each output store by one chunk so the SP queue never
            # parks on a store whose PV results aren't ready yet (an in-order
            # queue head would block all later Q/K loads behind it).
            if fin is not None:
                pending_fins.append(fin)
            if len(pending_fins) > 2:
                pending_fins.pop(0)()
            prev = (b, q0, w, et)
        if prev is not None:
            subs, fin = pv_parts(*prev)
            for s_ in subs:
                s_()
            pending_fins.append(fin)
        # Tail flush: the SP queue is idle by now and its descriptor path is
        # much faster than gpsimd SWDGE, so the final drain waits less.
        for f in pending_fins:
            f(nc.sync)

    nc.compile()
    return nc


def _get_compiled(extents):
    key = tuple(extents)
    if key not in _COMPILED:
        _COMPILED[key] = _build(key)
    return _COMPILED[key]


def _plan(valid_len):
    """Sort batches by valid_len desc into BPC slots of N_CORES batches.
    Returns (order, extents): order[s * N_CORES + c] = original batch index
    handled by core c in slot s; extents[s] = rounded max valid_len of the
    slot (0 means the whole slot is masked and fully host-filled)."""
    vl = np.asarray(valid_len).astype(np.int64)
    order = np.argsort(-vl, kind="stable")
    extents = []
    for s in range(BPC):
        block = vl[order[s * N_CORES:(s + 1) * N_CORES]]
        m = int(block.max())
        extents.append(min(S, -(-m // 128) * 128))
    return order, extents


def run_sharded(queries, keys, values, valid_len, **spmd_kwargs):
    """Run the kernel on 8 cores; returns (full_output, BassKernelResults)."""
    q = np.asarray(queries, dtype=np.float32)
    k = np.asarray(keys, dtype=np.float32)
    v = np.asarray(values, dtype=np.float32)
    vl = np.asarray(valid_len).astype(np.int64)

    order, extents = _plan(vl)
    if not any(extents):
        # Every query row in every batch is masked: the whole output is the
        # uniform-attention result; no device work needed.
        return np.broadcast_to(v.mean(axis=1)[:, None, :],
                               (B, S, D)).astype(np.float32).copy(), None
    nc = _get_compiled(extents)

    mask = (np.arange(S)[None, :] < vl[:, None]).astype(np.float32)  # [B, S]
    scale = np.float32(1.0 / np.sqrt(D))
    qm = q * (mask * scale)[:, :, None]
    qT = np.ascontiguousarray(qm.transpose(0, 2, 1)).astype(np.float16)
    kT = np.ascontiguousarray(k.transpose(0, 2, 1)).astype(np.float16)
    vA = np.concatenate([v, np.ones((B, S, 1), np.float32)], axis=2)
    vA = vA.astype(np.float16)                                # [B, S, D+1]

    in_maps = []
    for c in range(N_CORES):
        bsel = [int(order[s * N_CORES + c]) for s in range(BPC)]
        in_maps.append({
            "qT": np.ascontiguousarray(qT[bsel]),
            "kT": np.ascontiguousarray(kT[bsel]),
            "vA": np.ascontiguousarray(vA[bsel]),
        })
    res = run_bass_kernel_spmd(nc, in_maps, list(range(N_CORES)), **spmd_kwargs)

    # Rows beyond each slot's extent were skipped on device; they are exactly
    # the uniform-attention result mean(V) (reference: softmax of a constant
    # -100000 row is uniform).
    vmean = v.mean(axis=1)                                    # [B, D]
    full = np.empty((B, S, D), np.float32)
    for s in range(BPC):
        e = extents[s]
        for c in range(N_CORES):
            b = int(order[s * N_CORES + c])
            if e > 0:
                full[b, :e] = res.results[c]["out"][s, :e]
            if e < S:
                full[b, e:] = vmean[b]
    return full, res


def kernel(queries, keys, values, valid_len):
    out, _ = run_sharded(queries, keys, values, valid_len)
    return out



# revision 9
# speedup vs baseline: 1.4225x; 1.4225x over previous
"""Trainium2 Bass kernel for batched dot-product attention with query-row
masking (nn_DotProductAttention: B=32, Q=K=2048, D=128, fp32).

Strategy
--------
- The reference masks whole QUERY rows: rows q >= valid_len[b] get constant
  scores -> uniform softmax -> output row = mean(V). We fold the mask and
  1/sqrt(D) into Q on the host (masked query rows become zero queries ->
  zero scores -> exp(0)=1 -> uniform, exactly matching the reference).
- Work is counted in 128-row query tiles. ceil(valid_len/128) tiles per
  batch are bin-packed onto 8 cores x M segments: every core runs the same
  M-segment schedule (extents E_0..E_{M-1}, compiled per extents tuple);
  segment s of core c processes up to E_s rows of ONE batch (host-chosen
  gather; batches may be split across bins, K/V duplicated as needed).
  This balances per-core rows near ceil(total_tiles/8) instead of the
  sorted slot-max schedule (~12% fewer rows).
- Host prep: pre-transpose Q and K to [D, seq] layout, append a ones
  column to V; all three cast to fp16 (matmuls run at the full 1 cycle/row
  PE rate; fp16 keeps 10 mantissa bits). fp8 was measured to break the
  accuracy budget (dominant-softmax rows copy V's quant error to the
  output), so the PE path stays fp16.
- Device per segment: scores^T [k, q] via fp16 matmuls; exp of the scores
  is SPLIT between ScalarE (exact exp) and the otherwise-idle DVE, which
  computes a Schraudolph-style exp: bits = round(x*1024/ln2 + 15360)
  written as int16 == the fp16 bit pattern of 2^(x/ln2) (~3% rel err on a
  fraction of the weights; softmax renormalization cancels most of it).
  The e tiles then feed fp16 matmuls against [V | 1] accumulating P@V and
  the softmax denominator in one PSUM tile. DVE computes the denominator
  reciprocal and the normalizing PSUM->SBUF copyback. Softmax skips
  max-subtraction: scores are ~N(0,1) so exp never overflows fp32.
- DMA: K loads split into pieces over the SP (+ACT at kernel start) HWDGE
  queues so the PE starts early; V loads and output stores ride the gpsimd
  SWDGE queues (stores are emitted two chunks late so no in-order queue
  ever parks on a store whose PV results aren't ready). The next segment's
  K/V are prefetched a full segment ahead.
"""

import sys

for _p in ("/opt/trn_rl_repo", "/root/.axon_site/_ro/trn_rl_repo"):
    if _p not in sys.path:
        sys.path.append(_p)

import math
from contextlib import ExitStack

import numpy as np

import concourse.bacc as bacc
import concourse.tile as tile
from concourse import mybir
from concourse.bass_utils import run_bass_kernel_spmd

B, S, D = 32, 2048, 128
N_CORES = 8
NKT = S // 128              # k-tiles (keys are never masked)
F32 = mybir.dt.float32
F16 = mybir.dt.float16
I16 = mybir.dt.int16

# Fraction of exp psum-groups handled by DVE (Schraudolph) instead of ACT.
DVE_FRAC = 0.45
# Schraudolph constants: int16 bits = round(x * C1 + C2) == fp16(exp(x)).
# C2 is offset by -62 bits to cancel the systematic overestimate of the
# piecewise-linear 2^f (tuned numerically against the reference; the ACT
# tiles use exact exp, so an uncentered bias would not normalize away).
SCH_C1 = 1024.0 / math.log(2.0)
SCH_C2 = 15360.0 - 62.0

_COMPILED = {}


def _seg_widths(extent, first_seg, last_seg):
    """Decompose a segment's query extent into score-chunk widths (<=512)."""
    ws = []
    e = extent
    while e >= 512:
        ws.append(512)
        e -= 512
    for w in (256, 128):
        while e >= w:
            ws.append(w)
            e -= w
    if last_seg and ws and ws[-1] == 512:
        ws[-1:] = [256, 128, 128]
    return ws


def _build(caps):
    extents = [c * 128 for c in caps]
    nseg = len(extents)
    tot = sum(extents)
    nc = bacc.Bacc("TRN2", target_bir_lowering=False, debug=False,
                   num_devices=N_CORES)
    qT = nc.dram_tensor("qT", [D, tot], F16, kind="ExternalInput")
    kT = nc.dram_tensor("kT", [nseg, D, S], F16, kind="ExternalInput")
    vA = nc.dram_tensor("vA", [nseg, S, D + 1], F16, kind="ExternalInput")
    out = nc.dram_tensor("out", [tot, D], F32, kind="ExternalOutput")

    active = [s for s in range(nseg) if extents[s] > 0]
    seg_q0 = np.concatenate([[0], np.cumsum(extents)]).tolist()

    with tile.TileContext(nc) as tc, ExitStack() as ctx:
        qk_pool = ctx.enter_context(tc.tile_pool(name="qk", bufs=3))
        v_pool = ctx.enter_context(tc.tile_pool(name="v", bufs=3))
        e_pool = ctx.enter_context(tc.tile_pool(name="e", bufs=1))
        o_pool = ctx.enter_context(tc.tile_pool(name="o", bufs=5))
        r_pool = ctx.enter_context(tc.tile_pool(name="r", bufs=4))
        s_psum = ctx.enter_context(tc.tile_pool(name="sps", bufs=3, space="PSUM"))
        o_psum = ctx.enter_context(tc.tile_pool(name="ops", bufs=2, space="PSUM"))

        tiles = {}
        et_ctr = [0]                  # global round-robin for et slot tags
        dve_acc = [0.0]               # weighted round-robin for exp engine
        NKP = 4                       # kt is loaded in NKP column pieces

        def load_seg(b):
            # kt in pieces so the first score matmuls can start after a small
            # amount of DMA. For the first segment (nothing else running) the
            # pieces alternate between the SP and ACT HWDGE queues for 2x
            # bandwidth; later segments prefetch during compute on SP only.
            # V goes through the gpsimd SWDGE queues.
            kt = [qk_pool.tile([D, S // NKP], F16, name=f"kt{p}")
                  for p in range(NKP)]
            for p in range(NKP):
                eng = nc.scalar if (b == active[0] and p % 2 == 1) else nc.sync
                eng.dma_start(
                    out=kt[p],
                    in_=kT[b, :, p * (S // NKP):(p + 1) * (S // NKP)])
            vt = v_pool.tile([128, NKT, D + 1], F16, name="vt")
            half = NKT // 2
            nc.gpsimd.dma_start(
                out=vt[:, 0:half, :],
                in_=vA[b, 0:half * 128, :].rearrange("(t p) d -> p t d", p=128))
            nc.gpsimd.dma_start(
                out=vt[:, half:NKT, :],
                in_=vA[b, half * 128:S, :].rearrange("(t p) d -> p t d", p=128))
            tiles[b] = (kt, vt)

        def scores_chunk_groups(b, q0, w):
            """Emit the chunk's qt load now; return per-psum-group closures
            (each emits its score matmuls + one exp) and the et list the
            closures fill in. q0 is the global flat query column offset."""
            qt = qk_pool.tile([D, w], F16, name="qt", tag="qt")
            nc.sync.dma_start(out=qt, in_=qT[:, q0:q0 + w])
            if b not in tiles:
                load_seg(b)
            kt, vt = tiles[b]
            # Pack j's so each exp instruction covers a full [128, 1024]
            # 2-bank PSUM tile regardless of chunk width (fewer, wider
            # elementwise instructions amortize per-instruction overhead).
            jpt = max(1, 1024 // w)            # j's packed per psum tile
            et = [None] * NKT                  # per j: (tile, col offset)

            def make_group(jt):
                def g():
                  with tc.high_priority(offset=300):
                    j0 = jt * jpt
                    cnt = min(jpt, NKT - j0)
                    tw = w * cnt
                    s_ps = s_psum.tile([128, tw], F32, name="s_ps", tag="s_ps",
                                       padded_shape=[128, 1024])
                    for u in range(cnt):
                        j = j0 + u
                        kp, kc = j // (NKT // NKP), j % (NKT // NKP)
                        nc.tensor.matmul(
                            s_ps[:, u * w:u * w + w],
                            kt[kp][:, kc * 128:kc * 128 + 128],
                            qt,
                            start=True, stop=True)
                    slot = et_ctr[0] % 32
                    et_ctr[0] += 1
                    e = e_pool.tile([128, tw], F16, name=f"et{slot}",
                                    tag=f"et{slot}", padded_shape=[128, 1024])
                    dve_acc[0] += DVE_FRAC
                    if dve_acc[0] >= 1.0:
                        dve_acc[0] -= 1.0
                        nc.vector.tensor_scalar(
                            e.bitcast(I16), s_ps, SCH_C1, SCH_C2,
                            mybir.AluOpType.mult, mybir.AluOpType.add)
                    else:
                        nc.scalar.activation(e, s_ps,
                                             mybir.ActivationFunctionType.Exp,
                                             bias=0.0, scale=1.0)
                    for u in range(cnt):
                        et[j0 + u] = (e, u * w)
                return g

            return [make_group(jt) for jt in range((NKT + jpt - 1) // jpt)], et

        def pv_parts(b, q0, w, et):
            """Return per-q-subtile closures + a finalizer (output store)."""
            kt, vt = tiles[b]
            nst = w // 128
            o_sb = o_pool.tile([128, nst, D], F32, name="o_sb", tag="o_sb")

            def make_sub(t):
                def s():
                    o_ps = o_psum.tile([128, D + 1], F32, name="o_ps")
                    for j in range(NKT):
                        e, off = et[j]
                        nc.tensor.matmul(o_ps,
                                         e[:, off + t * 128:off + (t + 1) * 128],
                                         vt[:, j, :], start=(j == 0),
                                         stop=(j == NKT - 1))
                    rec = r_pool.tile([128, 1], F32, name="rec")
                    nc.vector.reciprocal(rec, o_ps[:, D:D + 1])
                    nc.vector.tensor_scalar_mul(o_sb[:, t, :], o_ps[:, 0:D], rec)
                return s

            def fin(eng=None):
                (eng or nc.gpsimd).dma_start(
                    out=out[q0:q0 + w, :].rearrange("(t p) d -> p t d", p=128),
                    in_=o_sb[:, 0:nst, :])

            return [make_sub(t) for t in range(nst)], fin

        pending_fins = []
        chunks = []
        for s in active:
            q0 = seg_q0[s]
            for w in _seg_widths(extents[s], s == active[0], s == active[-1]):
                chunks.append((s, q0, w))
                q0 += w

        # Emit score groups of chunk i interleaved with PV subtiles of chunk
        # i-1 (scores lead by ~2 groups) so the PE keeps feeding the exp
        # stream even across transitions to narrow chunks, instead of
        # running a long PV block while the exp engines starve.
        prev = None
        for i, (b, q0, w) in enumerate(chunks):
            groups, et = scores_chunk_groups(b, q0, w)
            # Prefetch upcoming segments' K/V two chunks ahead (the pools
            # are triple-buffered, so at most two segments ahead of the one
            # being consumed may be in flight).
            for j in (i + 1, i + 2):
                if j < len(chunks) and chunks[j][0] not in tiles:
                    if len(tiles) - active.index(b) < 3:
                        load_seg(chunks[j][0])
            subs, fin = pv_parts(*prev) if prev is not None else ([], None)
            G, T = len(groups), len(subs)
            a = bi = 0
            while a < G or bi < T:
                if a < G and (T == 0 or a * T <= (bi + 1) * G):
                    groups[a]()
                    a += 1
                else:
                    subs[bi]()
                    bi += 1
            # Delay each output store by one chunk so the SP queue never
            # parks on a store whose PV results aren't ready yet (an in-order
            # queue head would block all later Q/K loads behind it).
            if fin is not None:
                pending_fins.append(fin)
            if len(pending_fins) > 2:
                pending_fins.pop(0)()
            prev = (b, q0, w, et)
        if prev is not None:
            subs, fin = pv_parts(*prev)
            for s_ in subs:
                s_()
            pending_fins.append(fin)
        # Tail flush: the SP queue is idle by now and its descriptor path is
        # much faster than gpsimd SWDGE, so the final drain waits less.
        for f in pending_fins:
            f(nc.sync)

    nc.compile()
    return nc


def _get_compiled(extents):
    key = tuple(extents)
    if key not in _COMPILED:
        _COMPILED[key] = _build(key)
    return _COMPILED[key]


def _feasible(caps, needs):
    """Greedy check/packing: place batches (desc) into bins of given caps
    (8 of each cap), one batch per bin, batches splittable across bins.
    Returns list of (cap_index_bin, batch, tile_offset, count) or None."""
    bins = []                        # (cap, seg_index, core_slot) flattened
    for si, c in enumerate(caps):
        for _ in range(N_CORES):
            bins.append([c, si])
    bins.sort(key=lambda x: -x[0])
    items = sorted([(n, b) for b, n in needs.items() if n > 0], reverse=True)
    placed = []                      # (seg_index, batch, tile_off, cnt)
    free = bins[:]                   # descending caps
    for n, b in items:
        off = 0
        rem = n
        while rem > 0:
            if not free:
                return None
            if rem >= free[0][0]:
                cap, si = free.pop(0)
                placed.append((si, b, off, cap))
                off += cap
                rem -= cap
            else:
                # best fit: smallest cap >= rem
                idx = None
                for k in range(len(free) - 1, -1, -1):
                    if free[k][0] >= rem:
                        idx = k
                        break
                if idx is None:
                    return None
                cap, si = free.pop(idx)
                placed.append((si, b, off, rem))
                off += rem
                rem = 0
    return placed


def _plan(valid_len):
    """Bin-pack query tiles onto 8 cores x M segments. Returns
    (extents, assign) where assign[(core, seg)] = (batch, tile_off, cnt)."""
    vl = np.asarray(valid_len).astype(np.int64)
    needs = {b: int(-(-int(vl[b]) // 128)) for b in range(B)}
    T = sum(needs.values())
    if T == 0:
        return [], {}

    # Baseline fallback: sorted slot-max schedule (always feasible).
    order = np.argsort(-vl, kind="stable")
    base_caps = []
    for s in range(B // N_CORES):
        blk = vl[order[s * N_CORES:(s + 1) * N_CORES]]
        base_caps.append(int(-(-int(blk.max()) // 128)))
    base_caps = [c for c in base_caps if c > 0]

    best = (sum(base_caps), len(base_caps), tuple(base_caps))
    lo = -(-T // N_CORES)
    maxcap = max(needs.values())
    found = {tuple(base_caps): _feasible(base_caps, needs)}

    def gen_tuples(m, total):
        # descending tuples of length m summing to total, caps <= 16
        def rec(prefix, remaining, parts, cap):
            if parts == 1:
                if 1 <= remaining <= cap:
                    yield prefix + (remaining,)
                return
            for c in range(min(cap, remaining - (parts - 1)), 0, -1):
                if c * parts < remaining:
                    break
                yield from rec(prefix + (c,), remaining - c, parts - 1, c)
        yield from rec((), total, m, min(16, maxcap if maxcap > 0 else 16))

    done = False
    for total in range(lo, best[0]):
        for m in (4, 5, 6):
            for caps in gen_tuples(m, total):
                if caps[0] * 1 < maxcap and maxcap > 16:
                    continue
                pl = _feasible(list(caps), needs)
                if pl is not None:
                    best = (total, m, caps)
                    found[caps] = pl
                    done = True
                    break
            if done:
                break
        if done:
            break

    caps = list(best[2])
    placed = found[tuple(caps)]
    # Assign bins to cores: per segment tier, hand out bins round-robin.
    assign = {}
    seg_fill = {si: 0 for si in range(len(caps))}
    for si, b, off, cnt in placed:
        core = seg_fill[si]
        seg_fill[si] += 1
        assign[(core, si)] = (b, off, cnt)
    return caps, assign


def run_sharded(queries, keys, values, valid_len, **spmd_kwargs):
    """Run the kernel on 8 cores; returns (full_output, BassKernelResults)."""
    q = np.asarray(queries, dtype=np.float32)
    k = np.asarray(keys, dtype=np.float32)
    v = np.asarray(values, dtype=np.float32)
    vl = np.asarray(valid_len).astype(np.int64)

    caps, assign = _plan(vl)
    vmean = v.mean(axis=1)                                    # [B, D]
    if not caps:
        # Every query row in every batch is masked: the whole output is the
        # uniform-attention result; no device work needed.
        return np.broadcast_to(vmean[:, None, :],
                               (B, S, D)).astype(np.float32).copy(), None
    nc = _get_compiled(caps)
    nseg = len(caps)
    tot = sum(caps) * 128

    mask = (np.arange(S)[None, :] < vl[:, None]).astype(np.float32)  # [B, S]
    scale = np.float32(1.0 / np.sqrt(D))
    qm = q * (mask * scale)[:, :, None]
    qT = np.ascontiguousarray(qm.transpose(0, 2, 1)).astype(np.float16)
    kT = np.ascontiguousarray(k.transpose(0, 2, 1)).astype(np.float16)
    vA = np.concatenate([v, np.ones((B, S, 1), np.float32)], axis=2)
    vA = vA.astype(np.float16)                                # [B, S, D+1]

    seg_q0 = np.concatenate([[0], np.cumsum(np.asarray(caps) * 128)])
    in_maps = []
    for c in range(N_CORES):
        qTc = np.zeros((D, tot), np.float16)
        kTc = np.zeros((nseg, D, S), np.float16)
        vAc = np.zeros((nseg, S, D + 1), np.float16)
        vAc[:, :, D] = 1.0          # empty bins: avoid 0-denominator NaNs
        for si in range(nseg):
            ent = assign.get((c, si))
            if ent is None:
                continue
            b, off, cnt = ent
            r0, r1 = off * 128, min((off + cnt) * 128, S)
            col0 = int(seg_q0[si])
            qTc[:, col0:col0 + (r1 - r0)] = qT[b][:, r0:r1]
            kTc[si] = kT[b]
            vAc[si] = vA[b]
        in_maps.append({"qT": qTc, "kT": kTc, "vA": vAc})
    res = run_bass_kernel_spmd(nc, in_maps, list(range(N_CORES)), **spmd_kwargs)

    # Start from the uniform-attention fill (exact for fully masked rows),
    # then scatter the device rows for each placed piece (valid rows only).
    full = np.empty((B, S, D), np.float32)
    full[:] = vmean[:, None, :]
    for (c, si), (b, off, cnt) in assign.items():
        r0 = off * 128
        r1 = min((off + cnt) * 128, S)
        lim = int(vl[b])
        if r0 >= lim:
            continue
        r1v = min(r1, lim)
        col0 = int(seg_q0[si])
        full[b, r0:r1v] = res.results[c]["out"][col0:col0 + (r1v - r0)]
    return full, res


def kernel(queries, keys, values, valid_len):
    out, _ = run_sharded(queries, keys, values, valid_len)
    return out


# revision 17
# speedup vs baseline: 1.4873x; 1.0455x over previous
"""Trainium2 Bass kernel for batched dot-product attention with query-row
masking (nn_DotProductAttention: B=32, Q=K=2048, D=128, fp32).

Strategy
--------
- The reference masks whole QUERY rows: rows q >= valid_len[b] get constant
  scores -> uniform softmax -> output row = mean(V). We fold the mask and
  1/sqrt(D) into Q on the host (masked query rows become zero queries ->
  zero scores -> exp(0)=1 -> uniform, exactly matching the reference).
- Work is counted in 128-row query tiles. ceil(valid_len/128) tiles per
  batch are bin-packed onto 8 cores x M segments: every core runs the same
  M-segment schedule (extents E_0..E_{M-1}, compiled per extents tuple);
  segment s of core c processes up to E_s rows of ONE batch (host-chosen
  gather; batches may be split across bins, K/V duplicated as needed).
  This balances per-core rows near ceil(total_tiles/8) instead of the
  sorted slot-max schedule (~12% fewer rows).
- Host prep: pre-transpose Q and K to [D, seq] layout, append a ones
  column to V; all three cast to fp16 (matmuls run at the full 1 cycle/row
  PE rate; fp16 keeps 10 mantissa bits). fp8 was measured to break the
  accuracy budget (dominant-softmax rows copy V's quant error to the
  output), so the PE path stays fp16.
- Device per segment: scores^T [k, q] via fp16 matmuls; exp of the scores
  is SPLIT between ScalarE (exact exp) and the otherwise-idle DVE, which
  computes a Schraudolph-style exp: bits = round(x*1024/ln2 + 15360)
  written as int16 == the fp16 bit pattern of 2^(x/ln2) (~3% rel err on a
  fraction of the weights; softmax renormalization cancels most of it).
  The e tiles then feed fp16 matmuls against [V | 1] accumulating P@V and
  the softmax denominator in one PSUM tile. DVE computes the denominator
  reciprocal and the normalizing PSUM->SBUF copyback. Softmax skips
  max-subtraction: scores are ~N(0,1) so exp never overflows fp32.
- DMA: K loads split into pieces over the SP (+ACT at kernel start) HWDGE
  queues so the PE starts early; V loads and output stores ride the gpsimd
  SWDGE queues (stores are emitted two chunks late so no in-order queue
  ever parks on a store whose PV results aren't ready). The next segment's
  K/V are prefetched a full segment ahead.
"""

import sys

for _p in ("/opt/trn_rl_repo", "/root/.axon_site/_ro/trn_rl_repo"):
    if _p not in sys.path:
        sys.path.append(_p)

import math
from contextlib import ExitStack

import numpy as np

import concourse.bacc as bacc
import concourse.tile as tile
from concourse import mybir
from concourse.bass_utils import run_bass_kernel_spmd

B, S, D = 32, 2048, 128
N_CORES = 8
NKT = S // 128              # k-tiles (keys are never masked)
F32 = mybir.dt.float32
F16 = mybir.dt.float16
I16 = mybir.dt.int16

# Fraction of exp psum-groups handled by DVE (Schraudolph) instead of ACT.
DVE_FRAC = 0.43
# Dummy matmuls issued at kernel start: they run during the otherwise-dead
# initial DMA window and hold the PE p-state at full clock so the first
# real score matmuls don't pay the ~3x ramp penalty.
WARMUP_MM = 14
# Schraudolph constants: int16 bits = round(x * C1 + C2) == fp16(exp(x)).
# C2 is offset by -62 bits to cancel the systematic overestimate of the
# piecewise-linear 2^f (tuned numerically against the reference; the ACT
# tiles use exact exp, so an uncentered bias would not normalize away).
SCH_C1 = 1024.0 / math.log(2.0)
SCH_C2 = 15360.0 - 62.0

_COMPILED = {}


def _seg_widths(extent, first_seg, last_seg):
    """Decompose a segment's query extent into score-chunk widths (<=512)."""
    ws = []
    e = extent
    while e >= 512:
        ws.append(512)
        e -= 512
    for w in (256, 128):
        while e >= w:
            ws.append(w)
            e -= w
    if last_seg and ws and ws[-1] == 512:
        ws[-1:] = [256, 128, 128]
    return ws


def _build(caps):
    extents = [c * 128 for c in caps]
    nseg = len(extents)
    tot = sum(extents)
    nc = bacc.Bacc("TRN2", target_bir_lowering=False, debug=False,
                   num_devices=N_CORES)
    qT = nc.dram_tensor("qT", [D, tot], F16, kind="ExternalInput")
    kT = nc.dram_tensor("kT", [nseg, D, S], F16, kind="ExternalInput")
    vA = nc.dram_tensor("vA", [nseg, S, D + 1], F16, kind="ExternalInput")
    out = nc.dram_tensor("out", [tot, D], F32, kind="ExternalOutput")

    active = [s for s in range(nseg) if extents[s] > 0]
    seg_q0 = np.concatenate([[0], np.cumsum(extents)]).tolist()

    with tile.TileContext(nc) as tc, ExitStack() as ctx:
        qk_pool = ctx.enter_context(tc.tile_pool(name="qk", bufs=3))
        v_pool = ctx.enter_context(tc.tile_pool(name="v", bufs=3))
        e_pool = ctx.enter_context(tc.tile_pool(name="e", bufs=1))
        o_pool = ctx.enter_context(tc.tile_pool(name="o", bufs=4))
        r_pool = ctx.enter_context(tc.tile_pool(name="r", bufs=4))
        s_psum = ctx.enter_context(tc.tile_pool(name="sps", bufs=3, space="PSUM"))
        o_psum = ctx.enter_context(tc.tile_pool(name="ops", bufs=2, space="PSUM"))

        tiles = {}
        et_ctr = [0]                  # global round-robin for et slot tags
        dve_acc = [0.0]               # weighted round-robin for exp engine
        NKP = 4                       # kt is loaded in NKP column pieces

        # PE p-state warmup: garbage matmuls with no data dependencies (the
        # warm tile is memset by the otherwise-idle DVE) that execute during
        # the initial K/Q DMA window.
        warm = qk_pool.tile([D, 512], F16, name="warm", tag="warm")
        nc.vector.memset(warm, 0.0)
        for _ in range(WARMUP_MM):
            w_ps = s_psum.tile([128, 512], F32, name="s_ps", tag="s_ps",
                               padded_shape=[128, 1024])
            nc.tensor.matmul(w_ps, warm[:, 0:128], warm, start=True, stop=True)

        def load_seg(b):
            # kt in pieces so the first score matmuls can start after a small
            # amount of DMA. For the first segment (nothing else running) the
            # pieces alternate between the ACT and SP HWDGE queues for 2x
            # bandwidth (piece 0 on ACT, so it loads in parallel with the qt
            # load that is already queued ahead of it on SP); later segments
            # prefetch during compute on SP only.
            # V goes through the gpsimd SWDGE queues.
            kt = [qk_pool.tile([D, S // NKP], F16, name=f"kt{p}")
                  for p in range(NKP)]
            for p in range(NKP):
                eng = nc.scalar if (b == active[0] and p % 2 == 0) else nc.sync
                eng.dma_start(
                    out=kt[p],
                    in_=kT[b, :, p * (S // NKP):(p + 1) * (S // NKP)])
            vt = v_pool.tile([128, NKT, D + 1], F16, name="vt")
            half = NKT // 2
            nc.gpsimd.dma_start(
                out=vt[:, 0:half, :],
                in_=vA[b, 0:half * 128, :].rearrange("(t p) d -> p t d", p=128))
            nc.gpsimd.dma_start(
                out=vt[:, half:NKT, :],
                in_=vA[b, half * 128:S, :].rearrange("(t p) d -> p t d", p=128))
            tiles[b] = (kt, vt)

        def scores_chunk_groups(b, q0, w):
            """Emit the chunk's qt load now; return per-psum-group closures
            (each emits its score matmuls + one exp) and the et list the
            closures fill in. q0 is the global flat query column offset."""
            qt = qk_pool.tile([D, w], F16, name="qt", tag="qt")
            nc.sync.dma_start(out=qt, in_=qT[:, q0:q0 + w])
            if b not in tiles:
                load_seg(b)
            kt, vt = tiles[b]
            # Pack j's so each exp instruction covers a full [128, 1024]
            # 2-bank PSUM tile regardless of chunk width (fewer, wider
            # elementwise instructions amortize per-instruction overhead).
            jpt = max(1, 1024 // w)            # j's packed per psum tile
            et = [None] * NKT                  # per j: (tile, col offset)

            def make_group(jt):
                def g():
                  with tc.high_priority(offset=300):
                    j0 = jt * jpt
                    cnt = min(jpt, NKT - j0)
                    tw = w * cnt
                    s_ps = s_psum.tile([128, tw], F32, name="s_ps", tag="s_ps",
                                       padded_shape=[128, 1024])
                    for u in range(cnt):
                        j = j0 + u
                        kp, kc = j // (NKT // NKP), j % (NKT // NKP)
                        nc.tensor.matmul(
                            s_ps[:, u * w:u * w + w],
                            kt[kp][:, kc * 128:kc * 128 + 128],
                            qt,
                            start=True, stop=True)
                    slot = et_ctr[0] % 24
                    et_ctr[0] += 1
                    e = e_pool.tile([128, tw], F16, name=f"et{slot}",
                                    tag=f"et{slot}", padded_shape=[128, 1024])
                    dve_acc[0] += DVE_FRAC
                    if dve_acc[0] >= 1.0:
                        dve_acc[0] -= 1.0
                        nc.vector.tensor_scalar(
                            e.bitcast(I16), s_ps, SCH_C1, SCH_C2,
                            mybir.AluOpType.mult, mybir.AluOpType.add)
                    else:
                        nc.scalar.activation(e, s_ps,
                                             mybir.ActivationFunctionType.Exp,
                                             bias=0.0, scale=1.0)
                    for u in range(cnt):
                        et[j0 + u] = (e, u * w)
                return g

            return [make_group(jt) for jt in range((NKT + jpt - 1) // jpt)], et

        def pv_parts(b, q0, w, et):
            """Return per-q-subtile closures + a finalizer (output store)."""
            kt, vt = tiles[b]
            nst = w // 128
            o_sb = o_pool.tile([128, nst, D], F32, name="o_sb", tag="o_sb")

            def make_sub(t):
                def s():
                    o_ps = o_psum.tile([128, D + 1], F32, name="o_ps")
                    for j in range(NKT):
                        e, off = et[j]
                        nc.tensor.matmul(o_ps,
                                         e[:, off + t * 128:off + (t + 1) * 128],
                                         vt[:, j, :], start=(j == 0),
                                         stop=(j == NKT - 1))
                    rec = r_pool.tile([128, 1], F32, name="rec")
                    nc.vector.reciprocal(rec, o_ps[:, D:D + 1])
                    nc.vector.tensor_scalar_mul(o_sb[:, t, :], o_ps[:, 0:D], rec)
                return s

            def fin(eng=None):
                # Output stores ride the ACT HWDGE queue: after the first
                # segment's K pieces it carries nothing else, so a store
                # parked on a not-yet-ready o_sb blocks no loads (SP carries
                # Q/K, gpsimd carries V).
                (eng or nc.scalar).dma_start(
                    out=out[q0:q0 + w, :].rearrange("(t p) d -> p t d", p=128),
                    in_=o_sb[:, 0:nst, :])

            return [make_sub(t) for t in range(nst)], fin

        pending_fins = []
        chunks = []
        for s in active:
            q0 = seg_q0[s]
            for w in _seg_widths(extents[s], s == active[0], s == active[-1]):
                chunks.append((s, q0, w))
                q0 += w

        # Emit score groups of chunk i interleaved with PV subtiles of chunk
        # i-1 (scores lead by ~2 groups) so the PE keeps feeding the exp
        # stream even across transitions to narrow chunks, instead of
        # running a long PV block while the exp engines starve.
        prev = None
        for i, (b, q0, w) in enumerate(chunks):
            groups, et = scores_chunk_groups(b, q0, w)
            # Prefetch upcoming segments' K/V with at least ~1024 rows of
            # compute lead (a 1MB K+V load takes ~3us; the tail segments are
            # only 1-2 chunks long, so chunk-count lookahead is not enough).
            # The pools are triple-buffered: at most two segments ahead of
            # the one being consumed may be in flight.
            lead = 0
            for j in range(i + 1, len(chunks)):
                if lead >= 1024:
                    break
                if chunks[j][0] not in tiles:
                    if len(tiles) - active.index(b) < 3:
                        load_seg(chunks[j][0])
                    else:
                        break
                lead += chunks[j][2]
            subs, fin = pv_parts(*prev) if prev is not None else ([], None)
            G, T = len(groups), len(subs)
            a = bi = 0
            while a < G or bi < T:
                if a < G and (T == 0 or a * T <= (bi + 1) * G):
                    groups[a]()
                    a += 1
                else:
                    subs[bi]()
                    bi += 1
            # Emit the store one chunk late: the o_sb of chunk i-1 is ready
            # by then, so the ACT queue head never parks long.
            if fin is not None:
                pending_fins.append(fin)
            if len(pending_fins) > 1:
                pending_fins.pop(0)()
            prev = (b, q0, w, et)
        if prev is not None:
            subs, fin = pv_parts(*prev)
            for s_ in subs:
                s_()
            pending_fins.append(fin)
        # Final drain: spread the last stores over three otherwise-idle
        # HWDGE queues so they complete in parallel.
        for f, eng in zip(pending_fins, (nc.sync, nc.scalar, nc.gpsimd)):
            f(eng)

    nc.compile()
    return nc


def _get_compiled(extents):
    key = tuple(extents)
    if key not in _COMPILED:
        _COMPILED[key] = _build(key)
    return _COMPILED[key]


def _feasible(caps, needs):
    """Greedy check/packing: place batches (desc) into bins of given caps
    (8 of each cap), one batch per bin, batches splittable across bins.
    Returns list of (cap_index_bin, batch, tile_offset, count) or None."""
    bins = []                        # (cap, seg_index, core_slot) flattened
    for si, c in enumerate(caps):
        for _ in range(N_CORES):
            bins.append([c, si])
    bins.sort(key=lambda x: -x[0])
    items = sorted([(n, b) for b, n in needs.items() if n > 0], reverse=True)
    placed = []                      # (seg_index, batch, tile_off, cnt)
    free = bins[:]                   # descending caps
    for n, b in items:
        off = 0
        rem = n
        while rem > 0:
            if not free:
                return None
            if rem >= free[0][0]:
                cap, si = free.pop(0)
                placed.append((si, b, off, cap))
                off += cap
                rem -= cap
            else:
                # best fit: smallest cap >= rem
                idx = None
                for k in range(len(free) - 1, -1, -1):
                    if free[k][0] >= rem:
                        idx = k
                        break
                if idx is None:
                    return None
                cap, si = free.pop(idx)
                placed.append((si, b, off, rem))
                off += rem
                rem = 0
    return placed


def _plan(valid_len):
    """Bin-pack query tiles onto 8 cores x M segments. Returns
    (extents, assign) where assign[(core, seg)] = (batch, tile_off, cnt)."""
    vl = np.asarray(valid_len).astype(np.int64)
    needs = {b: int(-(-int(vl[b]) // 128)) for b in range(B)}
    T = sum(needs.values())
    if T == 0:
        return [], {}

    # Baseline fallback: sorted slot-max schedule (always feasible).
    order = np.argsort(-vl, kind="stable")
    base_caps = []
    for s in range(B // N_CORES):
        blk = vl[order[s * N_CORES:(s + 1) * N_CORES]]
        base_caps.append(int(-(-int(blk.max()) // 128)))
    base_caps = [c for c in base_caps if c > 0]

    best = (sum(base_caps), len(base_caps), tuple(base_caps))
    lo = -(-T // N_CORES)
    maxcap = max(needs.values())
    found = {tuple(base_caps): _feasible(base_caps, needs)}

    def gen_tuples(m, total):
        # descending tuples of length m summing to total, caps <= 16
        def rec(prefix, remaining, parts, cap):
            if parts == 1:
                if 1 <= remaining <= cap:
                    yield prefix + (remaining,)
                return
            for c in range(min(cap, remaining - (parts - 1)), 0, -1):
                if c * parts < remaining:
                    break
                yield from rec(prefix + (c,), remaining - c, parts - 1, c)
        yield from rec((), total, m, min(16, maxcap if maxcap > 0 else 16))

    done = False
    for total in range(lo, best[0]):
        for m in (4, 5, 6):
            for caps in gen_tuples(m, total):
                if caps[0] * 1 < maxcap and maxcap > 16:
                    continue
                pl = _feasible(list(caps), needs)
                if pl is not None:
                    best = (total, m, caps)
                    found[caps] = pl
                    done = True
                    break
            if done:
                break
        if done:
            break

    caps = list(best[2])
    placed = found[tuple(caps)]
    # Assign bins to cores: per segment tier, hand out bins round-robin.
    assign = {}
    seg_fill = {si: 0 for si in range(len(caps))}
    for si, b, off, cnt in placed:
        core = seg_fill[si]
        seg_fill[si] += 1
        assign[(core, si)] = (b, off, cnt)
    return caps, assign


def run_sharded(queries, keys, values, valid_len, **spmd_kwargs):
    """Run the kernel on 8 cores; returns (full_output, BassKernelResults)."""
    q = np.asarray(queries, dtype=np.float32)
    k = np.asarray(keys, dtype=np.float32)
    v = np.asarray(values, dtype=np.float32)
    vl = np.asarray(valid_len).astype(np.int64)

    caps, assign = _plan(vl)
    vmean = v.mean(axis=1)                                    # [B, D]
    if not caps:
        # Every query row in every batch is masked: the whole output is the
        # uniform-attention result; no device work needed.
        return np.broadcast_to(vmean[:, None, :],
                               (B, S, D)).astype(np.float32).copy(), None
    nc = _get_compiled(caps)
    nseg = len(caps)
    tot = sum(caps) * 128

    mask = (np.arange(S)[None, :] < vl[:, None]).astype(np.float32)  # [B, S]
    scale = np.float32(1.0 / np.sqrt(D))
    qm = q * (mask * scale)[:, :, None]
    qT = np.ascontiguousarray(qm.transpose(0, 2, 1)).astype(np.float16)
    kT = np.ascontiguousarray(k.transpose(0, 2, 1)).astype(np.float16)
    vA = np.concatenate([v, np.ones((B, S, 1), np.float32)], axis=2)
    vA = vA.astype(np.float16)                                # [B, S, D+1]

    seg_q0 = np.concatenate([[0], np.cumsum(np.asarray(caps) * 128)])
    in_maps = []
    for c in range(N_CORES):
        qTc = np.zeros((D, tot), np.float16)
        kTc = np.zeros((nseg, D, S), np.float16)
        vAc = np.zeros((nseg, S, D + 1), np.float16)
        vAc[:, :, D] = 1.0          # empty bins: avoid 0-denominator NaNs
        for si in range(nseg):
            ent = assign.get((c, si))
            if ent is None:
                continue
            b, off, cnt = ent
            r0, r1 = off * 128, min((off + cnt) * 128, S)
            col0 = int(seg_q0[si])
            qTc[:, col0:col0 + (r1 - r0)] = qT[b][:, r0:r1]
            kTc[si] = kT[b]
            vAc[si] = vA[b]
        in_maps.append({"qT": qTc, "kT": kTc, "vA": vAc})
    res = run_bass_kernel_spmd(nc, in_maps, list(range(N_CORES)), **spmd_kwargs)

    # Start from the uniform-attention fill (exact for fully masked rows),
    # then scatter the device rows for each placed piece (valid rows only).
    full = np.empty((B, S, D), np.float32)
    full[:] = vmean[:, None, :]
    for (c, si), (b, off, cnt) in assign.items():
        r0 = off * 128
        r1 = min((off + cnt) * 128, S)
        lim = int(vl[b])
        if r0 >= lim:
            continue
        r1v = min(r1, lim)
        col0 = int(seg_q0[si])
        full[b, r0:r1v] = res.results[c]["out"][col0:col0 + (r1v - r0)]
    return full, res


def kernel(queries, keys, values, valid_len):
    out, _ = run_sharded(queries, keys, values, valid_len)
    return out
